# revision 37
# baseline (speedup 1.0000x reference)
"""Trainium2 Bass kernel for BaselineFeedforwardNetwork recurrence.

Reference computation (per path, T=60 steps, serial in t):
    x_t = [features_t (8), delta_{t-1} (1)]            # (9,)
    h1  = relu(x_t @ W1 + b1)                          # (128,)
    h2  = relu(h1 @ W2 + b2)                           # (128,)
    d_t = h2 @ W3 + b3                                 # (1,)
Output: deltas (N, T).

Data-parallel over N=65536 paths across 8 NeuronCores (8192/core),
weights replicated, recurrence local per core.

Per-core dataflow (bf16 matmuls, fp32 PSUM):
- 16 path tiles of 512; activations transposed (feature/hidden dim on
  partitions, paths on free dim). Tile i = 4s+g -> pack s (0..3),
  row-group g (0..3).
- Features preloaded to SBUF in 15-step mega-chunks (1 DMA per chunk
  per row group) - dma_start issue costs ~1us of engine time, so DMA
  count per step must be ~1.
- mm1a: K=8 feature matmul, 4x row-tiled (groups at partitions 32g).
- mm1b (rank1_fb=True, default): the delta feedback is rank-1, so
  W1[8,:]^T d(t) with d = W3^T h2 + b3 equals (outer(W1[8],W3))^T h2(t)
  + W1[8]^T b3. mm1b therefore contracts a constant 128x128 rank-1
  matrix W1W3 directly against the PREVIOUS step's h2 tile (still in
  SBUF), and the b3 term folds into the h1 eviction bias (b1p). This
  cuts the cross-step critical chain from 6 stages
  (mm1->h1ev->mm2->h2ev->mm3->deltaev->mm1b) to 4 - mm3 and the delta
  eviction become pure output path - measured ~7% faster end to end.
  (rank1_fb=False falls back to the K=1 matmul against ds.)
- mm2: full K=128 matmul per tile.
- mm3: block-diagonal W3 (column 32g of a (128,97) slice of W3sp), so
  the 4 tiles of a pack accumulate their deltas into ONE psum bank at
  rows {0,32,64,96}; one delta eviction per pack stages the step output.
- defer_mm3=2 (default): with rank1_fb the mm3+delta-eviction group is
  pure output path, so it is issued 2 packs LATE in the in-order PE
  stream. This removes the per-pack PE stall that otherwise waits for
  pack s's h2 evictions to drain the VectorE/ScalarE queues before
  mm3(s) can issue (~3-5% measured).
- PSUM layout (default): ph1 4 banks + ph2 3 banks + pd 1 bank. The
  third ph2 bank (enabled by shrinking pd to 1, which the deferred
  output path tolerates) widens the mm2 bank-reuse window and measured
  ~9% faster (12.4 vs 13.65 us/step interleaved A/B).
- Evictions (PSUM->SBUF + bias + relu): split between ScalarE ACT and
  VectorE tensor_scalar to balance the two engines (the throughput
  bottleneck at ~712-743ns per (128,512) tile).
- One out-DMA per step writes the step's deltas to DRAM.
"""

import os
import sys

import numpy as np

for _p in ("/opt/trn_rl_repo", "/root/.axon_site/_ro/trn_rl_repo"):
    if _p not in sys.path and os.path.isdir(_p):
        sys.path.append(_p)

import ml_dtypes  # noqa: E402

N_FULL = 65536
T_FULL = 60
F = 8
HID = 128
N_CORES = 8
NSH = N_FULL // N_CORES  # 8192 paths per core

BF16 = ml_dtypes.bfloat16


def build_kernel(nsh=NSH, t_steps=T_FULL, num_cores=N_CORES, b3_value=0.0,
                 chunk_steps=15, pair_h1=False, pair_pd=False, merge_pd=False,
                 t_decl=None, repeats=1, col_mm3=False, ph2_bufs=3, pd_bufs=1,
                 ds_bufs=2, out_q="sync", ev_split="alt", h1_bufs=6, h2_bufs=4,
                 rank1_fb=True, defer_mm3=2):
    """Builds the per-core Bass graph. Returns the compiled nc.

    t_decl: declared time extent of the DRAM tensors (defaults to t_steps).
    repeats: run the whole t_steps loop R times inside one NEFF (timing only;
    the recurrence restarts at each repeat). Both knobs exist so the timing
    harness can compare identical-I/O NEFFs whose device time differs by a
    known step count, making the slope independent of dispatch/transfer cost.
    """
    import concourse.bass as bass
    import concourse.tile as tile
    from concourse import bacc, mybir

    if t_decl is None:
        t_decl = t_steps
    if defer_mm3:
        assert rank1_fb and not (merge_pd or col_mm3)
    if rank1_fb:
        h2_bufs = max(h2_bufs, 20)
    bf = mybir.dt.bfloat16
    f32 = mybir.dt.float32
    NT = 512                       # path-tile width (one fp32 psum bank)
    ntiles = nsh // NT
    npacks = ntiles // 4           # pack = 4 row-tiled tiles
    xw = npacks * NT               # per-step free width
    TC = min(chunk_steps, t_steps)
    assert ntiles % 4 == 0

    nc = bacc.Bacc(
        "TRN2", target_bir_lowering=False, debug=False,
        num_devices=num_cores,
    )

    feat = nc.declare_dram_parameter("features", [4, F, t_decl, xw], bf, isOutput=False)
    w1p = nc.declare_dram_parameter("W1p", [128, HID], bf, isOutput=False)
    w1d = nc.declare_dram_parameter("W1d", [128, HID], bf, isOutput=False)
    w2 = nc.declare_dram_parameter("W2", [HID, HID], bf, isOutput=False)
    w3sp = nc.declare_dram_parameter("W3sp", [128, 8, 113], bf, isOutput=False)
    if col_mm3:
        w3f = nc.declare_dram_parameter("W3f", [128, 97], bf, isOutput=False)
        w3l = nc.declare_dram_parameter("W3l", [128, 97], bf, isOutput=False)
        w3o = nc.declare_dram_parameter("W3o", [128, 1], bf, isOutput=False)
    if rank1_fb:
        w1w3 = nc.declare_dram_parameter("W1W3", [128, 128], bf, isOutput=False)
        b1p = nc.declare_dram_parameter("b1p", [128, 1], f32, isOutput=False)
    b1 = nc.declare_dram_parameter("b1", [128, 1], f32, isOutput=False)
    b2 = nc.declare_dram_parameter("b2", [128, 1], f32, isOutput=False)
    out = nc.declare_dram_parameter("out", [t_decl, 4, xw], bf, isOutput=True)

    Relu = mybir.ActivationFunctionType.Relu
    Copy = mybir.ActivationFunctionType.Copy
    add = mybir.AluOpType.add
    amax = mybir.AluOpType.max

    with tile.TileContext(nc) as tc:
        with (
            tc.tile_pool(name="consts", bufs=1) as cpool,
            tc.tile_pool(name="f", bufs=2) as fpool,
            tc.tile_pool(name="h1r", bufs=h1_bufs) as h1pool,
            tc.tile_pool(name="h2r", bufs=h2_bufs) as h2pool,
            tc.tile_pool(name="dst", bufs=ds_bufs) as dpool,
            tc.tile_pool(name="ph1", bufs=2 if pair_h1 else 4, space="PSUM") as ph1pool,
            tc.tile_pool(name="ph2", bufs=ph2_bufs, space="PSUM") as ph2pool,
            tc.tile_pool(name="pd", bufs=1 if pair_pd else pd_bufs, space="PSUM") as pdpool,
        ):
            w1sb = cpool.tile([128, HID], bf, tag="w1")
            w1dsb = cpool.tile([128, HID], bf, tag="w1d")
            w2sb = cpool.tile([HID, HID], bf, tag="w2")
            w3sb = cpool.tile([128, 8, 113], bf, tag="w3")
            if col_mm3:
                w3fsb = cpool.tile([128, 97], bf, tag="w3f")
                w3lsb = cpool.tile([128, 97], bf, tag="w3l")
                w3osb = cpool.tile([128, 1], bf, tag="w3o")
                nc.gpsimd.dma_start(w3fsb[:], w3f[:])
                nc.gpsimd.dma_start(w3lsb[:], w3l[:])
                nc.gpsimd.dma_start(w3osb[:], w3o[:])
            if rank1_fb:
                w1w3sb = cpool.tile([128, 128], bf, tag="w1w3")
                b1psb = cpool.tile([128, 1], f32, tag="b1p")
                nc.gpsimd.dma_start(w1w3sb[:], w1w3[:])
                nc.gpsimd.dma_start(b1psb[:], b1p[:])
            b1sb = cpool.tile([128, 1], f32, tag="b1")
            b2sb = cpool.tile([128, 1], f32, tag="b2")
            nc.gpsimd.dma_start(w1sb[:], w1p[:])
            nc.gpsimd.dma_start(w1dsb[:], w1d[:])
            nc.gpsimd.dma_start(w2sb[:], w2[:])
            nc.gpsimd.dma_start(w3sb[:], w3sp[:])
            nc.gpsimd.dma_start(b1sb[:], b1[:])
            nc.gpsimd.dma_start(b2sb[:], b2[:])

            def load_chunk(ci):
                t0 = ci * TC
                tl = min(TC, t_steps - t0)
                ftile = fpool.tile([128, TC * xw], bf, tag="f")
                for g in range(4):
                    nc.sync.dma_start(
                        ftile[32 * g:32 * g + 8, 0:tl * xw],
                        feat[g, 0:F, t0:t0 + tl, 0:xw],
                    )
                return ftile

            from collections import deque
            mm3_q = deque()
            fcur = load_chunk(0)
            fnxt = None
            ds_prev = None
            h2_prev = None
            nchunks = (t_steps + TC - 1) // TC

            for t in range(t_steps * repeats):
                u = t % t_steps
                tt = u % TC
                if tt == 0 and t > 0:
                    fcur = fnxt
                if tt == 0 and t + TC < t_steps * repeats:
                    fnxt = load_chunk((u // TC + 1) % nchunks)
                h2_cur = [[None] * 4 for _ in range(npacks)]

                if merge_pd:
                    ds_pairs = [dpool.tile([113, NT], bf, tag="dst",
                                           name=f"ds{t}_{p}")
                                for p in range(npacks // 2)]
                else:
                    ds = dpool.tile([128, xw], bf, tag="dst")
                pd = None
                for s in range(npacks):
                    # ---- mm1: K=8 features (+ K=1 delta), 4x row-tiled;
                    #      pairs (g0,g1) and (g2,g3) share a 2-bank psum ----
                    if pair_h1:
                        pairs = [ph1pool.tile([128, 2 * NT], f32, tag="ph1",
                                              name=f"ph1p{t}_{s}_{p}")
                                 for p in range(2)]
                        ph1ap = [pairs[g // 2][:, NT * (g % 2):NT * (g % 2 + 1)]
                                 for g in range(4)]
                    else:
                        pairs = [ph1pool.tile([128, NT], f32, tag="ph1",
                                              name=f"ph1p{t}_{s}_{p}")
                                 for p in range(4)]
                        ph1ap = [pairs[g][:] for g in range(4)]
                    for g in range(4):
                        fs = tt * xw + NT * s
                        nc.tensor.matmul(
                            ph1ap[g],
                            lhsT=w1sb[32 * g:32 * g + 8, :],
                            rhs=fcur[32 * g:32 * g + 8, fs:fs + NT],
                            start=True, stop=(t == 0),
                            tile_position=(32 * g, 0),
                        )
                    if u > 0:
                        for g in range(4):
                            if rank1_fb:
                                nc.tensor.matmul(
                                    ph1ap[g],
                                    lhsT=w1w3sb[:],
                                    rhs=h2_prev[s][g][:],
                                    start=False, stop=True,
                                )
                                continue
                            if merge_pd:
                                r = 32 * g + 16 * (s % 2)
                                lhsT_d = w1dsb[r:r + 1, :]
                                rhs_d = ds_prev[s // 2][r:r + 1, :]
                            else:
                                lhsT_d = w1dsb[32 * g:32 * g + 1, :]
                                rhs_d = ds_prev[32 * g:32 * g + 1, NT * s:NT * (s + 1)]
                            nc.tensor.matmul(
                                ph1ap[g],
                                lhsT=lhsT_d,
                                rhs=rhs_d,
                                start=False, stop=True,
                                tile_position=(32 * g, 0),
                            )
                    # h1 eviction: paired (one op per 2 tiles) or single
                    h1aps = []
                    if pair_h1:
                        for p in range(2):
                            h1r = h1pool.tile([128, 2 * NT], bf, tag="h1r",
                                              name=f"h1r{t}_{s}_{p}")
                            if p == 0:
                                nc.scalar.activation(h1r[:], pairs[p][:], Relu, bias=b1sb[:, 0:1])
                            else:
                                nc.vector.tensor_scalar(h1r[:], pairs[p][:], b1sb[:, 0:1], 0.0, add, amax)
                            h1aps += [h1r[:, 0:NT], h1r[:, NT:2 * NT]]
                    else:
                        b1use = b1psb if (rank1_fb and u > 0) else b1sb
                        for g in range(4):
                            h1r = h1pool.tile([128, NT], bf, tag="h1r",
                                              name=f"h1r{t}_{s}_{g}")
                            to_v = (g % 2 == 1) or (
                                ev_split == "bal2" and s == 0 and g == 0)
                            if not to_v:
                                nc.scalar.activation(h1r[:], pairs[g][:], Relu, bias=b1use[:, 0:1])
                            else:
                                nc.vector.tensor_scalar(h1r[:], pairs[g][:], b1use[:, 0:1], 0.0, add, amax)
                            h1aps.append(h1r[:])
                    if merge_pd:
                        if s % 2 == 0:
                            pd = pdpool.tile([113, NT], f32, tag="pd",
                                             name=f"pd{t}_{s}")
                        pdh = pd[:]
                    elif pair_pd and not defer_mm3:
                        if s % 2 == 0:
                            pd = pdpool.tile([97, 2 * NT], f32, tag="pd",
                                             name=f"pd{t}_{s}")
                        pdh = pd[:, NT * (s % 2):NT * (s % 2 + 1)]
                    elif not defer_mm3:
                        pd = pdpool.tile([97, NT], f32, tag="pd",
                                         name=f"pd{t}_{s}")
                        pdh = pd[:]
                    else:
                        pdh = None
                    for g in range(4):
                        ph2 = ph2pool.tile([128, NT], f32, tag="ph2")
                        nc.tensor.matmul(
                            ph2[:], lhsT=w2sb[:],
                            rhs=h1aps[g],
                            start=True, stop=True)
                        h2r = h2pool.tile([128, NT], bf, tag="h2r",
                                          name=f"h2r{t}_{s}_{g}")
                        h2_cur[s][g] = h2r
                        if g % 2 == 0:
                            nc.vector.tensor_scalar(h2r[:], ph2[:], b2sb[:, 0:1], 0.0, add, amax)
                        else:
                            nc.scalar.activation(h2r[:], ph2[:], Relu, bias=b2sb[:, 0:1])
                        # delta for tile (s,g) -> pd row 32g + 16*(s%2)
                        if merge_pd:
                            nc.tensor.matmul(
                                pdh,
                                lhsT=w3sb[:, 2 * g + (s % 2), :],
                                rhs=h2r[:],
                                start=(s % 2 == 0 and g == 0),
                                stop=(s % 2 == 1 and g == 3),
                            )
                        elif col_mm3:
                            if g == 0:
                                nc.tensor.matmul(
                                    pdh, lhsT=w3fsb[:], rhs=h2r[:],
                                    start=True, stop=False,
                                    tile_position=(0, 0))
                            elif g == 3:
                                nc.tensor.matmul(
                                    pdh, lhsT=w3lsb[:], rhs=h2r[:],
                                    start=False, stop=True,
                                    tile_position=(0, 0))
                            else:
                                nc.tensor.matmul(
                                    pdh[32 * g:32 * g + 1, :],
                                    lhsT=w3osb[:], rhs=h2r[:],
                                    start=False, stop=False,
                                    tile_position=(0, 32 * g))
                        elif not defer_mm3:
                            nc.tensor.matmul(
                                pdh,
                                lhsT=w3sb[:, 2 * g, 0:97],
                                rhs=h2r[:],
                                start=(g == 0), stop=(g == 3),
                            )
                    # delta eviction (+b3)
                    if merge_pd:
                        if s % 2 == 1:
                            p = s // 2
                            if p % 2 == 0:
                                nc.scalar.activation(ds_pairs[p][:], pd[:], Copy,
                                                     bias=float(b3_value))
                            else:
                                nc.vector.tensor_scalar(ds_pairs[p][:], pd[:],
                                                        float(b3_value), None, add)
                            nc.sync.dma_start(
                                out[u, 0:4, 1024 * p:1024 * (p + 1)],
                                ds_pairs[p][0:113:16, :],
                            )
                    elif pair_pd and not defer_mm3:
                        if s % 2 == 1:
                            dsl = ds[0:97, NT * (s - 1):NT * (s + 1)]
                            if s % 4 == 1:
                                nc.scalar.activation(dsl, pd[:], Copy, bias=float(b3_value))
                            else:
                                nc.vector.tensor_scalar(dsl, pd[:], float(b3_value), None, add)
                    elif not defer_mm3:
                        dsl = ds[0:97, NT * s:NT * (s + 1)]
                        to_act = (s % 2 == 0) if ev_split == "alt" else (s != 1)
                        if to_act:
                            nc.scalar.activation(dsl, pd[:], Copy, bias=float(b3_value))
                        else:
                            nc.vector.tensor_scalar(dsl, pd[:], float(b3_value), None, add)
                    elif not pair_pd:
                        def mk_mm3(t_=t, u_=u, s_=s, ds_=ds, h2l=list(h2_cur[s])):
                            def go():
                                pdq = pdpool.tile([97, NT], f32, tag="pd",
                                                  name=f"pdq{t_}_{s_}")
                                for g in range(4):
                                    nc.tensor.matmul(
                                        pdq[:], lhsT=w3sb[:, 2 * g, 0:97],
                                        rhs=h2l[g][:],
                                        start=(g == 0), stop=(g == 3))
                                dsl = ds_[0:97, NT * s_:NT * (s_ + 1)]
                                if s_ % 2 == 0:
                                    nc.scalar.activation(dsl, pdq[:], Copy,
                                                         bias=float(b3_value))
                                else:
                                    nc.vector.tensor_scalar(
                                        dsl, pdq[:], float(b3_value), None, add)
                                if s_ == npacks - 1:
                                    oq2 = nc.gpsimd if out_q == "gpsimd" else nc.sync
                                    oq2.dma_start(out[u_], ds_[0:97:32, :])
                            return go
                        mm3_q.append(mk_mm3())
                        while len(mm3_q) > defer_mm3:
                            mm3_q.popleft()()
                    elif s % 2 == 1:
                        def mk_mm3p(t_=t, u_=u, s_=s, ds_=ds,
                                    h2a=list(h2_cur[s - 1]), h2b=list(h2_cur[s])):
                            def go():
                                pdq = pdpool.tile([97, 2 * NT], f32, tag="pd",
                                                  name=f"pdq{t_}_{s_}")
                                for half, h2l in ((0, h2a), (1, h2b)):
                                    for g in range(4):
                                        nc.tensor.matmul(
                                            pdq[:, NT * half:NT * (half + 1)],
                                            lhsT=w3sb[:, 2 * g, 0:97],
                                            rhs=h2l[g][:],
                                            start=(g == 0), stop=(g == 3))
                                dsl = ds_[0:97, NT * (s_ - 1):NT * (s_ + 1)]
                                nc.scalar.activation(dsl, pdq[:], Copy,
                                                     bias=float(b3_value))
                                if s_ == npacks - 1:
                                    oq2 = nc.gpsimd if out_q == "gpsimd" else nc.sync
                                    oq2.dma_start(out[u_], ds_[0:97:32, :])
                            return go
                        mm3_q.append(mk_mm3p())
                        while len(mm3_q) > max(1, defer_mm3 // 2):
                            mm3_q.popleft()()
                if merge_pd:
                    ds_prev = ds_pairs
                else:
                    if not defer_mm3:
                        # one out-DMA for the whole step
                        oq = nc.gpsimd if out_q == "gpsimd" else nc.sync
                        oq.dma_start(out[u], ds[0:97:32, :])
                    ds_prev = ds
                h2_prev = h2_cur
            while mm3_q:
                mm3_q.popleft()()

    nc.compile()
    return nc


def build_kernel_v2(nsh=NSH, t_steps=T_FULL, num_cores=N_CORES, b3_value=0.0,
                    chunk_steps=15, t_decl=None, repeats=1,
                    ph1_bufs=2, ph2_mode="single", ph2_bufs=3, pd_bufs=1,
                    pd_mode="single", pipelined=False):
    """v2: fewer/larger PSUM evictions + col-tiled mm3 + software pipelining.

    Differences vs v1:
    - ph1/ph2 are (128,1024) 2-bank pairs: ONE FD=1024 eviction per 2 tiles
      (18 eviction ops/step vs 36), assigned greedily to VectorE/ScalarE by a
      projected-busy cost model (V=(120+FD)/0.96ns, A=(222+FD)/1.2ns).
    - mm3 is column-tiled per pack: an M=97 start=True matmul claims the pd
      bank (row 0 = g0 delta), M=1 matmuls at tile_position (0,32g) drop g1/g2
      deltas concurrently on distinct PE column groups, and an M=97 closer
      carries g3 + stop=True. Deltas for pack s land at pd[:, 512*(s%2)] of a
      (97,1024) pd pair -> ONE FD=1024 delta eviction per 2 packs (2/step).
    - ds is one (97, 2048) tile per step: row 32g, col 512s+c. mm1b reads
      ds[32g, 512s:512s+512] (engine APs must start at partition 0/32/64/96).
    - Software pipelining: mm1 of pack s+1 issues before mm2/mm3 of pack s so
      PE keeps the evictors fed.

    PSUM: ph1 2 pairs (4 banks) + ph2 1 pair (2) + pd 1 pair (2) = 8 banks.
    """
    import concourse.tile as tile
    from concourse import bacc, mybir

    if t_decl is None:
        t_decl = t_steps
    bf = mybir.dt.bfloat16
    f32 = mybir.dt.float32
    NT = 512
    ntiles = nsh // NT
    npacks = ntiles // 4
    xw = npacks * NT
    TC = min(chunk_steps, t_steps)
    assert npacks % 2 == 0

    nc = bacc.Bacc("TRN2", target_bir_lowering=False, debug=False,
                   num_devices=num_cores)

    feat = nc.declare_dram_parameter("features", [4, F, t_decl, xw], bf, isOutput=False)
    w1p = nc.declare_dram_parameter("W1p", [128, HID], bf, isOutput=False)
    w1d = nc.declare_dram_parameter("W1d", [128, HID], bf, isOutput=False)
    w2 = nc.declare_dram_parameter("W2", [HID, HID], bf, isOutput=False)
    w3f = nc.declare_dram_parameter("W3f", [128, 97], bf, isOutput=False)
    w3l = nc.declare_dram_parameter("W3l", [128, 97], bf, isOutput=False)
    w3o = nc.declare_dram_parameter("W3o", [128, 1], bf, isOutput=False)
    b1 = nc.declare_dram_parameter("b1", [128, 1], f32, isOutput=False)
    b2 = nc.declare_dram_parameter("b2", [128, 1], f32, isOutput=False)
    out = nc.declare_dram_parameter("out", [t_decl, 4, xw], bf, isOutput=True)

    Relu = mybir.ActivationFunctionType.Relu
    Copy = mybir.ActivationFunctionType.Copy
    add = mybir.AluOpType.add
    amax = mybir.AluOpType.max

    busy = {"V": 0.0, "A": 0.0}

    def ev_cost(eng, fd):
        # HW-measured back-to-back costs (evbench.py)
        return 359 + 1.22 * fd if eng == "V" else 361 + 0.96 * fd

    def evict(dst, src, fd, bias, relu, force=None):
        eng = force or min(("V", "A"), key=lambda e: busy[e] + ev_cost(e, fd))
        busy[eng] += ev_cost(eng, fd)
        if relu:
            if eng == "V":
                nc.vector.tensor_scalar(dst, src, bias, 0.0, add, amax)
            else:
                nc.scalar.activation(dst, src, Relu, bias=bias)
        else:
            if eng == "V":
                nc.vector.tensor_scalar(dst, src, bias, None, add)
            else:
                nc.scalar.activation(dst, src, Copy, bias=bias)

    with tile.TileContext(nc) as tc:
        with (
            tc.tile_pool(name="consts", bufs=1) as cpool,
            tc.tile_pool(name="f", bufs=2) as fpool,
            tc.tile_pool(name="h1r", bufs=4) as h1pool,
            tc.tile_pool(name="h2r", bufs=4) as h2pool,
            tc.tile_pool(name="dst", bufs=2) as dpool,
            tc.tile_pool(name="ph1", bufs=ph1_bufs, space="PSUM") as ph1pool,
            tc.tile_pool(name="ph2", bufs=ph2_bufs, space="PSUM") as ph2pool,
            tc.tile_pool(name="pd", bufs=pd_bufs, space="PSUM") as pdpool,
        ):
            w1sb = cpool.tile([128, HID], bf, tag="w1")
            w1dsb = cpool.tile([128, HID], bf, tag="w1d")
            w2sb = cpool.tile([HID, HID], bf, tag="w2")
            w3fsb = cpool.tile([128, 97], bf, tag="w3f")
            w3lsb = cpool.tile([128, 97], bf, tag="w3l")
            w3osb = cpool.tile([128, 1], bf, tag="w3o")
            if rank1_fb:
                w1w3sb = cpool.tile([128, 128], bf, tag="w1w3")
                b1psb = cpool.tile([128, 1], f32, tag="b1p")
                nc.gpsimd.dma_start(w1w3sb[:], w1w3[:])
                nc.gpsimd.dma_start(b1psb[:], b1p[:])
            b1sb = cpool.tile([128, 1], f32, tag="b1")
            b2sb = cpool.tile([128, 1], f32, tag="b2")
            nc.gpsimd.dma_start(w1sb[:], w1p[:])
            nc.gpsimd.dma_start(w1dsb[:], w1d[:])
            nc.gpsimd.dma_start(w2sb[:], w2[:])
            nc.gpsimd.dma_start(w3fsb[:], w3f[:])
            nc.gpsimd.dma_start(w3lsb[:], w3l[:])
            nc.gpsimd.dma_start(w3osb[:], w3o[:])
            nc.gpsimd.dma_start(b1sb[:], b1[:])
            nc.gpsimd.dma_start(b2sb[:], b2[:])

            def load_chunk(ci):
                t0 = ci * TC
                tl = min(TC, t_steps - t0)
                ftile = fpool.tile([128, TC * xw], bf, tag="f")
                for g in range(4):
                    nc.sync.dma_start(
                        ftile[32 * g:32 * g + 8, 0:tl * xw],
                        feat[g, 0:F, t0:t0 + tl, 0:xw],
                    )
                return ftile

            fcur = load_chunk(0)
            fnxt = None
            ds_prev = None
            nchunks = (t_steps + TC - 1) // TC

            for t in range(t_steps * repeats):
                u = t % t_steps
                tt = u % TC
                if tt == 0 and t > 0:
                    fcur = fnxt
                if tt == 0 and t + TC < t_steps * repeats:
                    fnxt = load_chunk((u // TC + 1) % nchunks)

                ds = dpool.tile([97, xw], bf, tag="dst", name=f"ds{t}")
                pd = None
                h1rs = [None] * npacks

                def mm1_stage(s):
                    fs = tt * xw + NT * s
                    prs = [ph1pool.tile([128, 2 * NT], f32, tag="ph1",
                                        name=f"ph1_{t}_{s}_{j}")
                           for j in range(2)]
                    for g in range(4):
                        nc.tensor.matmul(
                            prs[g // 2][:, NT * (g % 2):NT * (g % 2 + 1)],
                            lhsT=w1sb[32 * g:32 * g + 8, :],
                            rhs=fcur[32 * g:32 * g + 8, fs:fs + NT],
                            start=True, stop=(u == 0),
                            tile_position=(32 * g, 0),
                        )
                    if u > 0:
                        for g in range(4):
                            nc.tensor.matmul(
                                prs[g // 2][:, NT * (g % 2):NT * (g % 2 + 1)],
                                lhsT=w1dsb[32 * g:32 * g + 1, :],
                                rhs=ds_prev[32 * g:32 * g + 1, NT * s:NT * (s + 1)],
                                start=False, stop=True,
                                tile_position=(32 * g, 0),
                            )
                    hs = []
                    for j in range(2):
                        h1r = h1pool.tile([128, 2 * NT], bf, tag="h1r",
                                          name=f"h1r{t}_{s}_{j}")
                        evict(h1r[:], prs[j][:], 2 * NT, b1sb[:, 0:1], True)
                        hs.append(h1r)
                    h1rs[s] = hs

                def mm23_stage(s):
                    nonlocal pd
                    if pd_mode == "pair":
                        half = s % 2
                        if half == 0:
                            pd = pdpool.tile([97, 2 * NT], f32, tag="pd",
                                             name=f"pd{t}_{s // 2}")
                        pcol = slice(NT * half, NT * (half + 1))
                    else:
                        half = 1
                        pd = pdpool.tile([97, NT], f32, tag="pd",
                                         name=f"pd{t}_{s}")
                        pcol = slice(0, NT)
                    hs = h1rs[s]
                    h2s = []
                    for j in range(2):
                        h2r = h2pool.tile([128, 2 * NT], bf, tag="h2r",
                                          name=f"h2r{t}_{s}_{j}")
                        if ph2_mode == "pair":
                            ph2 = ph2pool.tile([128, 2 * NT], f32, tag="ph2",
                                               name=f"ph2_{t}_{s}_{j}")
                            for gi in range(2):
                                nc.tensor.matmul(
                                    ph2[:, NT * gi:NT * (gi + 1)], lhsT=w2sb[:],
                                    rhs=hs[j][:, NT * gi:NT * (gi + 1)],
                                    start=True, stop=True)
                            evict(h2r[:], ph2[:], 2 * NT, b2sb[:, 0:1], True)
                        else:
                            for gi in range(2):
                                ph2 = ph2pool.tile([128, NT], f32, tag="ph2",
                                                   name=f"ph2_{t}_{s}_{j}_{gi}")
                                nc.tensor.matmul(
                                    ph2[:], lhsT=w2sb[:],
                                    rhs=hs[j][:, NT * gi:NT * (gi + 1)],
                                    start=True, stop=True)
                                evict(h2r[:, NT * gi:NT * (gi + 1)], ph2[:],
                                      NT, b2sb[:, 0:1], True)
                        h2s.append(h2r)

                    for g in range(4):
                        rhs = h2s[g // 2][:, NT * (g % 2):NT * (g % 2 + 1)]
                        if g == 0:
                            nc.tensor.matmul(pd[0:97, pcol], lhsT=w3fsb[:],
                                             rhs=rhs, start=True, stop=False,
                                             tile_position=(0, 0))
                        elif g == 3:
                            nc.tensor.matmul(pd[0:97, pcol], lhsT=w3lsb[:],
                                             rhs=rhs, start=False, stop=True,
                                             tile_position=(0, 0))
                        else:
                            nc.tensor.matmul(pd[32 * g:32 * g + 1, pcol],
                                             lhsT=w3osb[:],
                                             rhs=rhs, start=False, stop=False,
                                             tile_position=(0, 32 * g))
                    if pd_mode == "pair":
                        if half == 1:
                            k = s // 2
                            evict(ds[0:97, 2 * NT * k:2 * NT * (k + 1)], pd[:],
                                  2 * NT, float(b3_value), False)
                    else:
                        evict(ds[0:97, NT * s:NT * (s + 1)], pd[:],
                              NT, float(b3_value), False)

                if pipelined:
                    # mm1(s) ahead of mm2/mm3(s-1)
                    for s in range(npacks + 1):
                        if s < npacks:
                            mm1_stage(s)
                        if s >= 1:
                            mm23_stage(s - 1)
                else:
                    for s in range(npacks):
                        mm1_stage(s)
                        mm23_stage(s)

                nc.sync.dma_start(out[u], ds[0:97:32, :])
                ds_prev = ds

    nc.compile()
    return nc


def prep_core_inputs_v2(features, W1, b1, W2, b2, W3, b3, num_cores=N_CORES):
    n, t_steps, f = features.shape
    nsh = n // num_cores
    NT = 512
    npacks = nsh // (4 * NT)
    xw = npacks * NT

    w1p = np.zeros((128, HID), dtype=BF16)
    w1d = np.zeros((128, HID), dtype=BF16)
    for g in range(4):
        w1p[32 * g:32 * g + 8, :] = W1[0:8].astype(BF16)
        w1d[32 * g, :] = W1[8].astype(BF16)
    w3f = np.zeros((128, 97), dtype=BF16)
    w3f[:, 0] = W3[:, 0].astype(BF16)
    w3l = np.zeros((128, 97), dtype=BF16)
    w3l[:, 96] = W3[:, 0].astype(BF16)
    w3o = W3[:, 0:1].astype(BF16)
    b3v = float(np.asarray(b3).reshape(-1)[0])
    w1w3 = np.outer(W3[:, 0].astype(np.float32),
                    W1[8].astype(np.float32)).astype(BF16)
    b1pv = (b1.reshape(-1).astype(np.float32)
            + W1[8].astype(np.float32) * b3v).reshape(128, 1)
    w2b = W2.astype(BF16)
    b1c = b1.reshape(128, 1).astype(np.float32)
    b2c = b2.reshape(128, 1).astype(np.float32)

    in_maps = []
    for c in range(num_cores):
        fc = features[c * nsh:(c + 1) * nsh]
        fpk = fc.reshape(npacks, 4, NT, t_steps, f)
        fpk = fpk.transpose(1, 4, 3, 0, 2).reshape(4, f, t_steps, xw)
        in_maps.append({
            "features": np.ascontiguousarray(fpk).astype(BF16),
            "W1p": w1p, "W1d": w1d, "W2": w2b, "W3f": w3f, "W3l": w3l,
            "W3o": w3o, "b1": b1c, "b2": b2c,
        })
    return in_maps


def gather_out_v2(res_core, nsh, t_steps):
    return gather_out(res_core, nsh, t_steps)


_NC_CACHE = {}


KERNEL_VERSION = int(os.environ.get("KERNEL_VERSION", "1"))


def _get_nc(nsh=NSH, t_steps=T_FULL, num_cores=N_CORES, b3_value=0.0,
            version=None, **bkw):
    if version is None:
        version = KERNEL_VERSION
    key = (version, nsh, t_steps, num_cores, float(b3_value),
           tuple(sorted(bkw.items())))
    if key not in _NC_CACHE:
        build = build_kernel_v2 if version == 2 else build_kernel
        _NC_CACHE[key] = build(nsh, t_steps, num_cores, b3_value, **bkw)
    return _NC_CACHE[key]


def prep_core_inputs(features, W1, b1, W2, b2, W3, b3, num_cores=N_CORES):
    """Host-side shard + repack. Returns list of per-core in_maps."""
    n, t_steps, f = features.shape
    nsh = n // num_cores
    NT = 512
    npacks = nsh // (4 * NT)
    xw = npacks * NT

    w1p = np.zeros((128, HID), dtype=BF16)
    w1d = np.zeros((128, HID), dtype=BF16)
    for g in range(4):
        w1p[32 * g:32 * g + 8, :] = W1[0:8].astype(BF16)
        w1d[32 * g, :] = W1[8].astype(BF16)
        w1d[32 * g + 16, :] = W1[8].astype(BF16)
    w3sp = np.zeros((128, 8, 113), dtype=BF16)
    for g in range(4):
        for u in range(2):
            w3sp[:, 2 * g + u, 32 * g + 16 * u] = W3[:, 0].astype(BF16)
    w3f = np.zeros((128, 97), dtype=BF16)
    w3f[:, 0] = W3[:, 0].astype(BF16)
    w3l = np.zeros((128, 97), dtype=BF16)
    w3l[:, 96] = W3[:, 0].astype(BF16)
    w3o = W3[:, 0:1].astype(BF16)
    b3v = float(np.asarray(b3).reshape(-1)[0])
    w1w3 = np.outer(W3[:, 0].astype(np.float32),
                    W1[8].astype(np.float32)).astype(BF16)
    b1pv = (b1.reshape(-1).astype(np.float32)
            + W1[8].astype(np.float32) * b3v).reshape(128, 1)
    w2b = W2.astype(BF16)
    b1c = b1.reshape(128, 1).astype(np.float32)
    b2c = b2.reshape(128, 1).astype(np.float32)

    in_maps = []
    for c in range(num_cores):
        fc = features[c * nsh:(c + 1) * nsh]          # (nsh, T, F)
        # path p = 2048s + 512g + c_ ; fpk[g, k, t, 512s + c_]
        fpk = fc.reshape(npacks, 4, NT, t_steps, f)   # (s, g, c_, t, k)
        fpk = fpk.transpose(1, 4, 3, 0, 2).reshape(4, f, t_steps, xw)
        in_maps.append({
            "features": np.ascontiguousarray(fpk).astype(BF16),
            "W1p": w1p, "W1d": w1d, "W2": w2b, "W3sp": w3sp,
            "W3f": w3f, "W3l": w3l, "W3o": w3o,
            "W1W3": w1w3, "b1p": b1pv,
            "b1": b1c, "b2": b2c,
        })
    return in_maps


def gather_out(res_core, nsh, t_steps):
    """(T, 4, xw) bf16 -> (nsh, T) fp32, path p = 2048s + 512g + c."""
    npacks = nsh // 2048
    o = np.asarray(res_core).astype(np.float32)       # (T, 4, xw)
    o = o.reshape(t_steps, 4, npacks, 512)            # (t, g, s, c)
    o = o.transpose(2, 1, 3, 0).reshape(nsh, t_steps)
    return o


def run(features, W1, b1, W2, b2, W3, b3, version=None, build_kwargs=None,
        **run_kwargs):
    """Run on the 8 cores; returns (full_output, BassKernelResults)."""
    from concourse.bass_utils import run_bass_kernel_spmd

    if version is None:
        version = KERNEL_VERSION
    features = np.asarray(features)
    n, t_steps, f = features.shape
    nsh = n // N_CORES
    prep = prep_core_inputs_v2 if version == 2 else prep_core_inputs
    in_maps = prep(features, W1, b1, W2, b2, W3, b3)
    nc = _get_nc(nsh, t_steps, N_CORES, float(np.asarray(b3).reshape(-1)[0]),
                 version=version, **(build_kwargs or {}))
    res = run_bass_kernel_spmd(nc, in_maps, core_ids=list(range(N_CORES)), **run_kwargs)
    gather = gather_out_v2 if version == 2 else gather_out
    outs = [gather(res.results[c]["out"], nsh, t_steps) for c in range(N_CORES)]
    return np.concatenate(outs, axis=0), res


def kernel(features, W1, b1, W2, b2, W3, b3):
    out, _ = run(features, W1, b1, W2, b2, W3, b3)
    return out

